# revision 30
# baseline (speedup 1.0000x reference)
"""Trainium2 Bass kernel for nn_Attention_37074157699274.

Multi-head self-attention over tiny 32-token groups:
  x [4, 1024, 32, 256] -> qkv -> per-(b,p)-group 8-head attention -> out proj.

Data-parallel over the 4096 (b,p) groups across 8 NeuronCores (512
groups/core); on-core, groups are processed in blocks of 4 (=128 tokens,
one partition span).  Design notes:
  - x is transposed + cast to bf16 on the HOST, so the kernel DMAs xT
    directly (512B-contiguous descriptors); no on-chip transpose of x.
    Weights are host-cast to bf16; output is stored bf16, upcast on host.
  - attn@v is computed TRANSPOSED (out [d, i] per (group, head)) via
    tile_position-packed 32-wide matmuls, yielding oT (inner-major)
    directly; no PE o-transpose.
  - Dependency tracking is tile-granular and cross-engine deps coarsen
    into per-engine counting semaphores, so whole-qkv evacuation runs as
    ONE ACT instruction (split evacs serialize through the counters), and
    q/k/v/oT tiles are sized to keep WAR chains off the critical cycle.
  - PSUM (8 banks): qkv tile 3, per-block att tile (dots pp0/pp1 + oT
    g-even/g-odd regions) 2x2 double-buffered, out-proj accumulator 1.
  - attn@v's 4 group-tiles drain into only 2 banks, so they run in two
    waves with the full-array out-proj of an older block dep-forced
    between them (PE NOPs carrying tile reads): the (0,0) 128x128
    out-proj tile drains wave-1's subarray tiles before wave-2 loads,
    so concurrent PSUM drains never hit the same bank+partitions.
  - x/out DMAs ride HWDGE on the otherwise-idle SP engine; softmax
    normalize runs on Pool; bias is added by the DVE store (STT).
  - Steady state is ~98% PE-bound at ~2.8us per 128-token block
    (qkv 3072 + dots 1024 + attn@v 1024 + out-proj 1024 PE cycles).

The V_* switches were used for schedule tuning; defaults are the
measured-best configuration.
"""

import os
import numpy as np
import ml_dtypes

# Tuning switches (sweep harness; final values hardcoded as defaults)
V_QV = int(os.environ.get("KV_QV", "2"))      # 0: v,q separate ACT; 1: merged qv ACT
V_OT = int(os.environ.get("KV_OT", "1"))      # 0: single DVE; 1: ACT w1 + DVE w2; 2: single ACT
V_MUL = int(os.environ.get("KV_MUL", "0"))    # 0: Pool; 1: DVE
V_OPC = int(os.environ.get("KV_OPC", "2"))    # 0: ACT; 1: DVE
V_EMIT = int(os.environ.get("KV_EMIT", "0"))  # 0: a first; 1: dots/softmax first
V_ORD = int(os.environ.get("KV_ORD", "0"))    # 0: v,q,k mm order; 1: q,v,k
V_NOP = int(os.environ.get("KV_NOP", "1"))    # 0: skip dep nops (debug)
V_DBG = int(os.environ.get("KV_DBG", "0"))    # debug: 1 qkv only, 2 +dots/sm, 3 +avT
V_WAVE = int(os.environ.get("KV_WAVE", "0"))  # 1: two-wave avT in att(i) + dep nop

import concourse.bacc as bacc
import concourse.bass as bass
from concourse import bass_utils, mybir
from concourse.tile import TileContext

F32 = mybir.dt.float32
BF16 = mybir.dt.bfloat16
AF = mybir.ActivationFunctionType
ALU = mybir.AluOpType
AX = mybir.AxisListType

B, P, N, DIM = 4, 1024, 32, 256
HEADS, DH, INNER = 8, 64, 512
SCALE = DH ** -0.5
NCORES = 8
GROUPS = B * P                   # 4096 independent attention groups
GPC = GROUPS // NCORES           # 512 groups per core
BLK = 128                        # tokens per block = 4 groups
GPB = BLK // N                   # 4 groups per block

# att tile column map (per block x):
#   [pp0 dots 0..128 | g0 oT 128..256 | g2' oT 384..512 ||
#    pp1 dots 512..640 | g1 oT 640..768 | g3' oT 896..1024]
# where g2'/g3' are the oT drains of the PREVIOUS block: attn@v of block x
# writes g0,g1 into att(x) and g2,g3 into att(x+1), so its four concurrent
# PE sub-array tiles drain into four physically distinct PSUM banks.
OT_SELF = {0: 128, 1: 640}
OT_NEXT = {2: 384, 3: 896}


def build_kernel_body(tc, xt_d, wqkv_d, wout_d, bout_d, out_d, nblk):
    nc = tc.nc

    with tc.tile_pool(name="wpool", bufs=1) as wp:
        # W_qkv bf16 [256, 1536] -> [128 part, dchunk 2, 1536]
        wqkv_b = wp.tile([128, 2, 3 * INNER], BF16, name="wqkv_b")
        nc.sync.dma_start(out=wqkv_b, in_=wqkv_d.rearrange("(c p) f -> p c f", c=2))
        # W_out bf16 [512, 256] -> [128 part, chunk 4, 256]
        wout_b = wp.tile([128, 4, DIM], BF16, name="wout_b")
        nc.sync.dma_start(out=wout_b, in_=wout_d.rearrange("(c p) f -> p c f", c=4))
        # bias folded into the out-proj accumulation via a rank-1 matmul:
        # ones[1,128] (lhsT) x bias[1,256] (rhs) adds bias[f] to every token.
        bias_f = wp.tile([1, DIM], F32, name="bias_f")
        nc.sync.dma_start(out=bias_f, in_=bout_d.unsqueeze(0))
        bias_row = wp.tile([1, DIM], BF16, name="bias_row")
        nc.vector.tensor_copy(bias_row, bias_f)
        ones1 = wp.tile([1, 128], BF16, name="ones1")
        nc.vector.memset(ones1, 1.0)
        bias_t = wp.tile([128, DIM], F32, name="bias_t")
        nc.sync.dma_start(out=bias_t,
                          in_=bout_d.unsqueeze(0).broadcast_to([128, DIM]))
        _main_loop(tc, xt_d, out_d, nblk, wqkv_b, wout_b, bias_row, ones1,
                   bias_t)


def _main_loop(tc, xt_d, out_d, nblk, wqkv_b, wout_b, bias_row, ones1,
                   bias_t):
    nc = tc.nc
    assert nblk % 2 == 0
    npair = nblk // 2
    # xT dram [256, tok]: row d = c*128+p; DMA view per pair of blocks
    xv = xt_d.rearrange("(c p) (j t) -> j p c t", c=2, t=2 * BLK)
    ov2 = out_d.rearrange("(n b p) d -> n p b d", b=2, p=BLK)

    with (
        tc.tile_pool(name="io", bufs=4) as iop,
        tc.tile_pool(name="work", bufs=4) as wk,
        tc.tile_pool(name="ps_qkv", bufs=1, space="PSUM") as pqkv,
        tc.tile_pool(name="ps_att", bufs=2, space="PSUM") as patt,
        tc.tile_pool(name="ps_op", bufs=1, space="PSUM") as pop,
    ):
        st = {}      # cross-stage tile refs keyed by block index

        def pe_dep_nop(hint, aps):
            # a PE NoOp whose ins carry tile reads: the dep annotator runs at
            # add_instruction time, so the reads must be present up front.
            if not V_NOP:
                return None
            ins = [nc.tensor.lower_ap(ap) for ap in aps]
            return nc.tensor.add_instruction(
                mybir.InstNoOp(
                    name=nc.get_next_instruction_name(),
                    text_hint=hint,
                    ins=ins,
                    outs=[]))

        def stage_a(i):
            # ---- xT load: one HWDGE DMA per 2 blocks, prefetched 2 pairs ----
            if i % 2 == 0:
                j = i // 2
                if j == 0:
                    for jj in (0, 1):
                        t = iop.tile([128, 2, 2 * BLK], BF16, tag="xT2",
                                     name="xT2")
                        nc.sync.dma_start(out=t, in_=xv[jj])
                        st["xT", jj] = t
                if j + 2 < npair:
                    t = iop.tile([128, 2, 2 * BLK], BF16, tag="xT2", name="xT2")
                    nc.sync.dma_start(out=t, in_=xv[j + 2])
                    st["xT", j + 2] = t
            xT2 = st["xT", i // 2]
            xT = xT2[:, :, BLK * (i % 2):BLK * (i % 2) + BLK]

            qkv_ps = pqkv.tile([128, 3 * INNER], F32, tag="qkv_ps", name="qkv_ps")

            def mm_v():
                for dc in range(2):
                    nc.tensor.matmul(
                        qkv_ps[:, 1024:1536],
                        lhsT=xT[:, dc],
                        rhs=wqkv_b[:, dc, 2 * INNER:3 * INNER],
                        start=(dc == 0), stop=(dc == 1))

            def mm_qk(cs):
                # q,k feature-major (chunk c = features 128c..128c+128)
                for c in cs:
                    for dc in range(2):
                        nc.tensor.matmul(
                            qkv_ps[:, 128 * c:128 * c + 128],
                            lhsT=wqkv_b[:, dc, 128 * c:128 * c + 128],
                            rhs=xT[:, dc],
                            start=(dc == 0), stop=(dc == 1))

            if V_ORD == 0:
                mm_v()
                mm_qk(range(8))
            elif V_ORD == 1:
                mm_qk(range(4))
                mm_v()
                mm_qk(range(4, 8))
            else:
                mm_qk(range(8))
                mm_v()
            # separate destination tiles per evac instr: a shared tile would
            # serialize the ACT and DVE evacs through a tile-granular WAW.
            if V_QV == 5:
                # qk + v as two ACT instrs: same engine (no cross-engine
                # serialization), but exp can slot between them
                qkv_sb = wk.tile([128, 3, 512], BF16, tag="qkv_sb",
                                 name="qkv_sb")
                nc.scalar.copy(qkv_sb[:, 0:2], qkv_ps.rearrange(
                    "p (h x) -> p h x", h=3)[:, 0:2])
                nc.scalar.copy(qkv_sb[:, 2], qkv_ps[:, 1024:1536])
                q_sb = qkv_sb[:, 0]
                k_sb = qkv_sb[:, 1]
                v_sb = qkv_sb[:, 2]
            elif V_QV == 4:
                # qk in one early ACT evac (dots' inputs), v on DVE (only
                # needed by attn@v a round later)
                qk_sb = wk.tile([128, 1024], BF16, tag="qk_sb", name="qk_sb")
                v_sb = wk.tile([128, 512], BF16, tag="v_sb", name="v_sb")
                nc.scalar.copy(qk_sb, qkv_ps[:, 0:1024])
                nc.vector.tensor_copy(v_sb, qkv_ps[:, 1024:1536])
                q_sb = qk_sb[:, 0:512]
                k_sb = qk_sb[:, 512:1024]
            elif V_QV == 2:
                # single ACT evac of the whole qkv block: avoids the
                # cross-engine serial chain the coarse engine-counter
                # semaphores impose on split evacs
                qkv_sb = wk.tile([128, 3, 512], BF16, tag="qkv_sb",
                                 name="qkv_sb")
                nc.scalar.copy(qkv_sb,
                               qkv_ps.rearrange("p (h x) -> p h x", h=3))
                q_sb = qkv_sb[:, 0]
                k_sb = qkv_sb[:, 1]
                v_sb = qkv_sb[:, 2]
            elif V_QV == 3:
                # single DVE evac
                qkv_sb = wk.tile([128, 3, 512], BF16, tag="qkv_sb",
                                 name="qkv_sb")
                nc.vector.tensor_copy(
                    qkv_sb, qkv_ps.rearrange("p (h x) -> p h x", h=3))
                q_sb = qkv_sb[:, 0]
                k_sb = qkv_sb[:, 1]
                v_sb = qkv_sb[:, 2]
            elif V_QV == 0:
                k_sb = wk.tile([128, 512], BF16, tag="k_sb", name="k_sb")
                v_sb = wk.tile([128, 512], BF16, tag="v_sb", name="v_sb")
                q_sb = wk.tile([128, 512], BF16, tag="q_sb", name="q_sb")
                nc.scalar.copy(v_sb, qkv_ps[:, 1024:1536])
                nc.scalar.copy(q_sb, qkv_ps[:, 0:512])
                nc.vector.tensor_copy(k_sb, qkv_ps[:, 512:1024])
            else:
                k_sb = wk.tile([128, 512], BF16, tag="k_sb", name="k_sb")
                qv_sb = wk.tile([128, 2, 512], BF16, tag="qv_sb", name="qv_sb")
                qv_in = qkv_ps.rearrange("p (h x) -> p h x", h=3)[:, 0:3:2]
                nc.scalar.copy(qv_sb, qv_in)
                q_sb = qv_sb[:, 0]
                v_sb = qv_sb[:, 1]
                nc.vector.tensor_copy(k_sb, qkv_ps[:, 512:1024])
            st["q", i] = q_sb
            st["k", i] = k_sb
            st["v", i] = v_sb

        def stage_dots(i):
            q_sb = st["q", i]
            k_sb = st["k", i]
            att = patt.tile([128, 1024], F32, tag="att", name="att")
            # dots per (g, h) 32x32; pp parity picks the bank (cols 0 / 512)
            for h in range(HEADS):
                c, pp = h // 2, h % 2
                for g in range(GPB):
                    col = 128 * c + 32 * g
                    dcol = 512 * pp + 32 * c
                    nc.tensor.matmul(
                        att[32 * g:32 * g + 32, dcol:dcol + 32],
                        lhsT=q_sb[64 * pp:64 * pp + 64, col:col + 32],
                        rhs=k_sb[64 * pp:64 * pp + 64, col:col + 32],
                        start=True, stop=True,
                        tile_position=(64 * pp, 32 * g))
            st["att", i] = att

        def stage_softmax(i):
            att = st["att", i]
            # em [128, 2, 128] bf16: [:, pp, 32c+j] = exp(dots) of head 2c+pp
            em = wk.tile([128, 2, 128], BF16, tag="em", name="em")
            nc.scalar.activation(
                em, att.rearrange("p (gb x) -> p gb x", gb=2)[:, :, 0:128],
                AF.Exp, bias=0.0, scale=SCALE)
            # softmax over j segments; a-index = 4pp + c  <->  head h = 2c+pp
            s_t = wk.tile([128, 8], F32, tag="s_t", name="s_t")
            nc.vector.reduce_sum(
                s_t, em.rearrange("p b (c j) -> p (b c) j", c=4), axis=AX.X)
            attn_b = wk.tile([128, 256], BF16, tag="attn_b", name="attn_b")
            if V_MUL == 2:
                # direct divide: skips the reciprocal hop
                nc.gpsimd.tensor_tensor(
                    out=attn_b.rearrange("p (a j) -> p a j", a=8),
                    in0=em.rearrange("p b (c j) -> p (b c) j", c=4),
                    in1=s_t.unsqueeze(2).broadcast_to([128, 8, 32]),
                    op=ALU.divide)
            elif V_MUL == 3:
                nc.vector.tensor_tensor(
                    out=attn_b.rearrange("p (a j) -> p a j", a=8),
                    in0=em.rearrange("p b (c j) -> p (b c) j", c=4),
                    in1=s_t.unsqueeze(2).broadcast_to([128, 8, 32]),
                    op=ALU.divide)
            else:
                r_t = wk.tile([128, 8], F32, tag="r_t", name="r_t")
                nc.vector.reciprocal(r_t, s_t)
                mul_eng = nc.gpsimd if V_MUL == 0 else nc.vector
                mul_eng.tensor_tensor(
                    out=attn_b.rearrange("p (a j) -> p a j", a=8),
                    in0=em.rearrange("p b (c j) -> p (b c) j", c=4),
                    in1=r_t.unsqueeze(2).broadcast_to([128, 8, 32]),
                    op=ALU.mult)
            # 32x32 block transpose: [(g,i),(a,j)] -> [(g,j),(a,i)]
            attnT = wk.tile([128, 256], BF16, tag="attnT", name="attnT")
            nc.vector.transpose(attnT, attn_b)
            st["attnT", i] = attnT

        def stage_avT(i):
            # attn@v transposed: out [d 64, i 32] per (g, h) -> oT directly.
            attnT = st["attnT", i]
            att = st["att", i]
            v_sb = st["v", i]

            def wave(gs, base):
                for c in range(4):
                    for pp in range(2):
                        h = 2 * c + pp
                        a = 4 * pp + c
                        for g in gs:
                            out = base[g][:, OT_W[g] + 32 * c:
                                          OT_W[g] + 32 * c + 32]
                            nc.tensor.matmul(
                                out[64 * pp:64 * pp + 64],
                                lhsT=v_sb[32 * g:32 * g + 32,
                                          64 * h:64 * h + 64],
                                rhs=attnT[32 * g:32 * g + 32,
                                          32 * a:32 * a + 32],
                                start=True, stop=True,
                                tile_position=(32 * g, 64 * pp))

            if V_WAVE:
                # both waves inside att(i); a dep-carrying NOP between them
                # guarantees wave-1 drains (sem) before wave-2 loads.
                OT_W = {0: 128, 1: 640, 2: 256, 3: 768}
                base = {g: att for g in range(4)}
                wave((0, 1), base)
                pe_dep_nop("avt_ser", [att[:, 128:256]])
                wave((2, 3), base)
            else:
                # g0,g1 -> att(i); g2,g3 -> att(i+1): four distinct PSUM
                # banks, all 32 matmuls may drain concurrently.  Safety of
                # the att(i+1) writes vs dots(i+1) comes from the WAR sem on
                # exp(i+1)'s read.
                att_n = st["att", i + 1]
                OT_W = dict(OT_SELF)
                OT_W.update(OT_NEXT)
                base = {0: att, 1: att, 2: att_n, 3: att_n}
                wave(range(4), base)

        def stage_oT_evac(i):
            att = st["att", i]
            att_n = st["att", i + 1] if not V_WAVE else att
            # oT_sb [128, c 4, tok 128] bf16 : [64(h%2)+d][h//2][32g+ii]
            # g = gb + 2*slot; tok offset = 64*slot + 32*gb + ii.
            # slot 0 (g0,g1) lives in att(i) cols 128..256/640..768;
            # slot 1 (g2,g3) in att(i+1) cols 384..512/896..1024.
            oT_sb = wk.tile([128, 4, 128], BF16, tag="oT_sb", name="oT_sb")
            ovv = oT_sb.rearrange("p c (slot gb ii) -> p gb slot c ii",
                                  slot=2, gb=2)
            inv_a = (att.rearrange("p (gb x) -> p gb x", gb=2)[:, :, 128:256]
                     .rearrange("p gb (c ii) -> p gb c ii", c=4))
            bcols = slice(256, 384) if V_WAVE else slice(384, 512)
            inv_b = (att_n.rearrange("p (gb x) -> p gb x", gb=2)[:, :, bcols]
                     .rearrange("p gb (c ii) -> p gb c ii", c=4))
            if V_OT == 2:
                nc.scalar.copy(ovv[:, :, 0], inv_a)
                nc.scalar.copy(ovv[:, :, 1], inv_b)
            else:
                nc.vector.tensor_copy(ovv[:, :, 0], inv_a)
                nc.vector.tensor_copy(ovv[:, :, 1], inv_b)
            st["oT", i] = oT_sb
            del st["attnT", i]
            del st["q", i]
            del st["k", i]
            del st["v", i]
            if i > 0:
                st.pop(("att", i - 1), None)

        def stage_out(i, op_ps):
            oT_sb = st.pop(("oT", i))
            if V_OPC == 2:
                for c in range(4):
                    nc.tensor.matmul(
                        op_ps, lhsT=oT_sb[:, c], rhs=wout_b[:, c],
                        start=(c == 0), stop=(c == 3))
            else:
                for c in range(4):
                    nc.tensor.matmul(
                        op_ps, lhsT=oT_sb[:, c], rhs=wout_b[:, c],
                        start=(c == 0), stop=False)
                nc.tensor.matmul(op_ps, lhsT=ones1, rhs=bias_row,
                                 start=False, stop=True)
            if i % 2 == 0:
                st["osb"] = iop.tile([128, 2, DIM], BF16, tag="out_sb2",
                                     name="out_sb2")
            if V_OPC == 0:
                nc.scalar.copy(st["osb"][:, i % 2], op_ps)
            elif V_OPC == 1:
                nc.vector.tensor_copy(st["osb"][:, i % 2], op_ps)
            else:
                nc.vector.scalar_tensor_tensor(
                    out=st["osb"][:, i % 2], in0=op_ps, scalar=1.0,
                    in1=bias_t, op0=ALU.mult, op1=ALU.add)
            if i % 2 == 1:
                nc.sync.dma_start(out=ov2[i // 2], in_=st["osb"])

        if V_DBG:
            for r in range(-1, nblk + 2):
                if 0 <= r + 1 < nblk:
                    stage_a(r + 1)
                if V_DBG >= 2 and 0 <= r < nblk:
                    stage_dots(r)
                    stage_softmax(r)
                if r == nblk - 1:
                    st["att", nblk] = patt.tile([128, 1024], F32, tag="att",
                                                name="att")
                if V_DBG >= 3 and 0 <= r - 1 < nblk:
                    stage_avT(r - 1)
                    if V_DBG >= 4:
                        stage_oT_evac(r - 1)
                i = r - 1
                if 0 <= i < nblk:
                    # dummy out so every block writes something
                    if i % 2 == 0:
                        st["osb"] = iop.tile([128, 2, DIM], BF16,
                                             tag="out_sb2", name="out_sb2")
                    dbg_src = (st[("oT", i)][:, 0:2].rearrange("p a b -> p (a b)")
                               if V_DBG >= 4 else st["q", i][:, 0:256])
                    nc.vector.tensor_copy(st["osb"][:, i % 2], dbg_src)
                    if i % 2 == 1:
                        nc.sync.dma_start(out=ov2[i // 2], in_=st["osb"])
                    if V_DBG < 3:
                        pass
                    if V_DBG == 2:
                        st.pop(("att", i), None)
                        st.pop(("attnT", i), None)
                    if V_DBG >= 3:
                        st.pop(("oT", i), None)
            return

        for r in range(-1, nblk + 2):
            if V_EMIT == 1:
                if 0 <= r < nblk:
                    stage_dots(r)
                    stage_softmax(r)
                if 0 <= r + 1 < nblk:
                    stage_a(r + 1)
            else:
                if 0 <= r + 1 < nblk:
                    stage_a(r + 1)
                if 0 <= r < nblk:
                    stage_dots(r)
                    if V_EMIT != 2:
                        stage_softmax(r)
            if r == nblk - 1:
                # dummy att(nblk): drain target for the last block's g2,g3
                st["att", nblk] = patt.tile([128, 1024], F32, tag="att",
                                            name="att")
            have_avt = 0 <= r - 1 < nblk
            have_op = 0 <= r - 2 < nblk
            if V_EMIT == 2 and have_avt:
                stage_avT(r - 1)
                stage_oT_evac(r - 1)
            if V_EMIT == 2 and 0 <= r < nblk:
                stage_softmax(r)
            if V_EMIT != 2 and have_avt:
                stage_avT(r - 1)
                stage_oT_evac(r - 1)
            if have_op:
                op_ps = pop.tile([128, DIM], F32, tag="op_ps", name="op_ps")
                stage_out(r - 2, op_ps)


def build(nblk):
    nc = bacc.Bacc("TRN2", target_bir_lowering=False, debug=False,
                   enable_asserts=False)
    tok = nblk * BLK
    xt_d = nc.dram_tensor("xt", [DIM, tok], BF16, kind="ExternalInput").ap()
    wqkv_d = nc.dram_tensor("w_qkv", [DIM, 3 * INNER], BF16,
                            kind="ExternalInput").ap()
    wout_d = nc.dram_tensor("w_out", [INNER, DIM], BF16,
                            kind="ExternalInput").ap()
    bout_d = nc.dram_tensor("b_out", [DIM], F32, kind="ExternalInput").ap()
    out_d = nc.dram_tensor("out", [tok, DIM], BF16, kind="ExternalOutput").ap()
    with TileContext(nc) as tc:
        build_kernel_body(tc, xt_d, wqkv_d, wout_d, bout_d, out_d, nblk)
    nc.compile()
    return nc


_NC_CACHE = {}


def _get_nc(nblk):
    if nblk not in _NC_CACHE:
        _NC_CACHE[nblk] = build(nblk)
    return _NC_CACHE[nblk]


def kernel(x, W_qkv, W_out, b_out, trace=False):
    assert x.shape == (B, P, N, DIM)
    nblk = GPC * N // BLK        # 128 blocks/core
    tok = nblk * BLK
    nc = _get_nc(nblk)
    bf16 = ml_dtypes.bfloat16
    xf = np.asarray(x, np.float32).reshape(NCORES, tok, DIM)
    wq = np.ascontiguousarray(np.asarray(W_qkv, np.float32)).astype(bf16)
    wo = np.ascontiguousarray(np.asarray(W_out, np.float32)).astype(bf16)
    bo = np.asarray(b_out, np.float32)
    in_maps = [
        {"xt": np.ascontiguousarray(xf[i].T).astype(bf16),
         "w_qkv": wq, "w_out": wo, "b_out": bo}
        for i in range(NCORES)
    ]
    res = bass_utils.run_bass_kernel_spmd(
        nc, in_maps, core_ids=list(range(NCORES)), trace=trace)
    out = np.concatenate([np.asarray(res.results[i]["out"], np.float32)
                          for i in range(NCORES)], axis=0)
    out = out.reshape(B, P, N, DIM)
    if trace:
        return out, res
    return out
